# revision 18
# baseline (speedup 1.0000x reference)
"""ASTRA contrastive loss on 8 Trainium2 NeuronCores (Bass/Tile), v7.

Pure data parallel: B=1024 sharded 128 samples/core (one SBUF partition
per sample). Embeddings staged bf16 (host cast; rel-err ~3e-6 vs 2e-2
budget). Per agent-group (ramped sizes so compute starts early and DMA
stays ahead):
    DVE  prod = mut*heal (bf16 TT, 2x)              into t3[:, 0, :, :]
    ACT  mut^2, heal^2 big-slab Squares             into t3[:, 1:3, :, :]
    DVE  radix-2 fold tree along D on [128, 3, ga, D], one instr per
         level for all 3 stats at 2x, then merged tensor_reduce ->
         st[128, 3, N] f32
Epilogue: idx/mask precompute hoisted into DMA dead time (packed into
ONE small DMA); slim post chain (relu(cos+1) == cos+1 since |cos|<=1,
folded into one tensor_scalar). Host sums per-core (contrib, valid).

Measured architecture facts (this deployment):
 - DVE @0.96GHz, 128 lanes; TT 2x needs all-bf16 packed operands;
   tensor_reduce/pool always 1x; ~130ns fixed cost per DVE instr;
   ACT @1.2GHz 1x squares; no ACT->PSUM accumulate. DVE floor
   ~536c/agent (product 128 + 3-stat folds 408) => ~36us pure.
 - Fold work can't leave DVE: PE ingest (128 elem/cyc either port)
   is slower than DVE 2x folds, and diag/[1,F] PSUM evacuation kills
   every matmul formulation. GpSimd poisons DVE via shared SBUF port.
 - Tail ~11us is runtime postamble (~255 per-semaphore reset instrs
   across 5 engines + out-DMA queue quiesce + exit barriers); roughly
   constant, counted in exec time. Routing the tiny output DMA through
   GpSimd software DGE (1 queue, no 16-queue sem trickle) bought ~1us.
 - Run-to-run variance: mode ~62-63us with occasional +10us outliers
   for the same NEFF; A/B only with >=3 runs, compare modes.
 - Tile-scheduler perturbations dominate micro-opts: merged-mh DMAs,
   epi-first triggers, merged squares, and st16 single-reduce each
   measured SLOWER (64->67-69us) despite lower nominal instr counts.
   This exact structure measured 63.9us; keep deltas surgical.
"""

import sys

import numpy as np

_REPO = "/opt/trn_rl_repo"
if _REPO not in sys.path:
    sys.path.insert(0, _REPO)

B, N, D = 1024, 64, 256
NCORES = 8
BP = B // NCORES          # samples per core (one SBUF partition each)
GROUP_SIZES = [6, 10, 16, 16, 16]  # ramped: small first group starts
                                   # compute ASAP; DMA stays ahead after
FOLD_STOP = 16
MARGIN = 1.0
ALPHA = 0.7
EPS = 1e-8

_NC_CACHE = {}


def _build_nc(reps=1):
    """Build the single-core Bass/Tile program (SPMD across 8 cores)."""
    from contextlib import ExitStack

    import concourse.bacc as bacc
    import concourse.tile as tile
    from concourse import mybir

    bf16 = mybir.dt.bfloat16
    f32 = mybir.dt.float32
    Alu = mybir.AluOpType
    Act = mybir.ActivationFunctionType

    nc = bacc.Bacc(None, target_bir_lowering=False, debug=False, num_devices=NCORES)
    mut_d = nc.declare_dram_parameter("emb_mut", [BP, N, D], bf16, isOutput=False)
    heal_d = nc.declare_dram_parameter("emb_heal", [BP, N, D], bf16, isOutput=False)
    # packed epilogue inputs: [:, 0]=idx_f, [:, 1:65]=mask_f, [:, 65:129]=iota_f
    epi_d = nc.declare_dram_parameter("epi_in", [BP, 2 * N + 1], f32, isOutput=False)
    out_d = nc.declare_dram_parameter("out", [BP, 2], f32, isOutput=True)

    def emit_body(tc, ctx, pools):
        (mut_pool, heal_pool, t3_pool, st_pool, ep_pool) = pools

        # stats, stat-major: [sample, {dot, ssm, ssh}, agent] f32
        st = st_pool.tile([BP, 3, N], f32, tag="st")
        st16 = st_pool.tile([BP, 3, N, FOLD_STOP], bf16, tag="st16")

        # ---- DMA triggers: first big group FIRST so its transfer wins
        # the bandwidth race; small epilogue DMA follows; then the rest.
        gsls = []
        a0 = 0
        for ga in GROUP_SIZES:
            gsls.append(slice(a0, a0 + ga))
            a0 += ga

        mts, hts = [], []
        mt = mut_pool.tile([BP, GROUP_SIZES[0], D], bf16, tag="mt0")
        nc.sync.dma_start(out=mt[:, :, :], in_=mut_d[:, gsls[0], :])
        mts.append(mt)
        ht = heal_pool.tile([BP, GROUP_SIZES[0], D], bf16, tag="ht0")
        nc.sync.dma_start(out=ht[:, :, :], in_=heal_d[:, gsls[0], :])
        hts.append(ht)

        epi_t = ep_pool.tile([BP, 2 * N + 1], f32, tag="epi")
        nc.sync.dma_start(out=epi_t[:, :], in_=epi_d[:, :])

        for g in range(1, len(GROUP_SIZES)):
            ga = GROUP_SIZES[g]
            mt = mut_pool.tile([BP, ga, D], bf16, tag=f"mt{g}")
            nc.sync.dma_start(out=mt[:, :, :], in_=mut_d[:, gsls[g], :])
            mts.append(mt)
            ht = heal_pool.tile([BP, ga, D], bf16, tag=f"ht{g}")
            nc.sync.dma_start(out=ht[:, :, :], in_=heal_d[:, gsls[g], :])
            hts.append(ht)

        idx_t = epi_t[:, 0:1]
        mask_t = epi_t[:, 1:N + 1]
        iota_t = epi_t[:, N + 1:2 * N + 1]

        out_sb = ep_pool.tile([BP, 2], f32, tag="outsb")

        # ---- epilogue precompute in the pre-loop DMA dead time ----
        idx_c = ep_pool.tile([BP, 1], f32, tag="idxc")
        nc.vector.tensor_scalar(out=idx_c[:, :], in0=idx_t, scalar1=0.0,
                                scalar2=float(N - 1), op0=Alu.max, op1=Alu.min)
        # valid = (idx == clip(idx)) -> directly into out column 1
        nc.vector.tensor_scalar(out=out_sb[:, 1:2], in0=idx_t,
                                scalar1=idx_c[:, 0:1], scalar2=None,
                                op0=Alu.is_equal)
        onehot = ep_pool.tile([BP, N], f32, tag="onehot")
        nc.vector.tensor_scalar(out=onehot[:, :], in0=iota_t,
                                scalar1=idx_c[:, 0:1], scalar2=None,
                                op0=Alu.is_equal)
        nmask = ep_pool.tile([BP, N], f32, tag="nmask")
        nc.vector.tensor_tensor(out=nmask[:, :], in0=mask_t,
                                in1=onehot[:, :], op=Alu.subtract)
        nc.vector.tensor_scalar(out=nmask[:, :], in0=nmask[:, :], scalar1=0.0,
                                scalar2=None, op0=Alu.max)
        cnt = ep_pool.tile([BP, 1], f32, tag="cnt")
        nc.vector.tensor_reduce(out=cnt[:, :], in_=nmask[:, :],
                                axis=mybir.AxisListType.X, op=Alu.add)
        cnt1 = ep_pool.tile([BP, 1], f32, tag="cnt1")
        nc.vector.tensor_scalar(out=cnt1[:, :], in0=cnt[:, :], scalar1=1.0,
                                scalar2=None, op0=Alu.max)
        icnt = ep_pool.tile([BP, 1], f32, tag="icnt")
        nc.vector.reciprocal(out=icnt[:, :], in_=cnt1[:, :])
        # (cnt>0) * (1-ALPHA): zero-count gate with the 0.3 weight folded in
        gposs = ep_pool.tile([BP, 1], f32, tag="gposs")
        nc.vector.tensor_scalar(out=gposs[:, :], in0=cnt[:, :], scalar1=0.0,
                                scalar2=1.0 - ALPHA, op0=Alu.is_gt,
                                op1=Alu.mult)

        # ---- main loop ----
        for g, ga in enumerate(GROUP_SIZES):
            mt, ht, gsl = mts[g], hts[g], gsls[g]
            # merged work tile, stat-major: [:,0]=prod, [:,1]=mut^2, [:,2]=heal^2
            t3 = t3_pool.tile([BP, 3, ga, D], bf16, tag=f"t3{g % 3}")
            nc.vector.tensor_tensor(out=t3[:, 0, :, :], in0=mt[:, :, :],
                                    in1=ht[:, :, :], op=Alu.mult)
            nc.scalar.activation(out=t3[:, 1, :, :], in_=mt[:, :, :],
                                 func=Act.Square)
            nc.scalar.activation(out=t3[:, 2, :, :], in_=ht[:, :, :],
                                 func=Act.Square)

            # DVE radix-2 fold tree along D: one instruction per level
            # covers all 3 stats (bf16 packed -> 2x mode)
            w = D // 2
            while w > FOLD_STOP:
                nc.vector.tensor_tensor(out=t3[:, :, :, 0:w],
                                        in0=t3[:, :, :, 0:w],
                                        in1=t3[:, :, :, w:2 * w], op=Alu.add)
                w //= 2
            nc.vector.tensor_tensor(out=st16[:, :, gsl, :],
                                    in0=t3[:, :, :, 0:FOLD_STOP],
                                    in1=t3[:, :, :, FOLD_STOP:2 * FOLD_STOP],
                                    op=Alu.add)

        nc.vector.tensor_reduce(out=st[:, :, :], in_=st16[:, :, :, :],
                                axis=mybir.AxisListType.X, op=Alu.add)

        dot = st[:, 0, :]
        ssm = st[:, 1, :]
        ssh = st[:, 2, :]

        # ---- per-sample epilogue, [128, 64] / [128, 1] f32 ----
        # cos = dot / sqrt(max(ssm*ssh, eps^4))
        den2 = ep_pool.tile([BP, N], f32, tag="den2")
        nc.vector.tensor_tensor(out=den2[:, :], in0=ssm, in1=ssh, op=Alu.mult)
        nc.vector.tensor_scalar(out=den2[:, :], in0=den2[:, :],
                                scalar1=EPS * EPS * EPS * EPS, scalar2=None,
                                op0=Alu.max)
        den = ep_pool.tile([BP, N], f32, tag="den")
        nc.scalar.activation(out=den[:, :], in_=den2[:, :], func=Act.Sqrt)
        rden = ep_pool.tile([BP, N], f32, tag="rden")
        nc.vector.reciprocal(out=rden[:, :], in_=den[:, :])
        cos = ep_pool.tile([BP, N], f32, tag="cos")
        nc.vector.tensor_tensor(out=cos[:, :], in0=dot, in1=rden[:, :],
                                op=Alu.mult)

        # cos at target; relu(cos_t + margin) == cos_t + 1 since |cos|<=1,
        # folded with the ALPHA weight: loss_t = ALPHA*(cos_t + MARGIN)
        ct_prod = ep_pool.tile([BP, N], f32, tag="ctprod")
        nc.vector.tensor_tensor(out=ct_prod[:, :], in0=cos[:, :],
                                in1=onehot[:, :], op=Alu.mult)
        cos_t = ep_pool.tile([BP, 1], f32, tag="cost")
        nc.vector.tensor_reduce(out=cos_t[:, :], in_=ct_prod[:, :],
                                axis=mybir.AxisListType.X, op=Alu.add)
        loss_t = ep_pool.tile([BP, 1], f32, tag="losst")
        nc.vector.tensor_scalar(out=loss_t[:, :], in0=cos_t[:, :],
                                scalar1=MARGIN, scalar2=ALPHA,
                                op0=Alu.add, op1=Alu.mult)

        # weighted loss_others: (1-ALPHA)*gate*(cnt - sum nmask*cos)/max(cnt,1)
        mc_prod = ep_pool.tile([BP, N], f32, tag="mcprod")
        nc.vector.tensor_tensor(out=mc_prod[:, :], in0=nmask[:, :],
                                in1=cos[:, :], op=Alu.mult)
        mc = ep_pool.tile([BP, 1], f32, tag="mc")
        nc.vector.tensor_reduce(out=mc[:, :], in_=mc_prod[:, :],
                                axis=mybir.AxisListType.X, op=Alu.add)
        so = ep_pool.tile([BP, 1], f32, tag="so")
        nc.vector.tensor_tensor(out=so[:, :], in0=cnt[:, :], in1=mc[:, :],
                                op=Alu.subtract)
        lo = ep_pool.tile([BP, 1], f32, tag="lo")
        nc.vector.tensor_tensor(out=lo[:, :], in0=so[:, :], in1=icnt[:, :],
                                op=Alu.mult)
        nc.vector.tensor_tensor(out=lo[:, :], in0=lo[:, :], in1=gposs[:, :],
                                op=Alu.mult)

        per = ep_pool.tile([BP, 1], f32, tag="per")
        nc.vector.tensor_tensor(out=per[:, :], in0=loss_t[:, :], in1=lo[:, :],
                                op=Alu.add)
        nc.vector.tensor_tensor(out=out_sb[:, 0:1], in0=per[:, :],
                                in1=out_sb[:, 1:2], op=Alu.mult)
        nc.gpsimd.dma_start(out=out_d[:, :], in_=out_sb[:, :])

    with tile.TileContext(nc) as tc, ExitStack() as ctx:
        pools = (
            ctx.enter_context(tc.tile_pool(name="mut", bufs=1)),
            ctx.enter_context(tc.tile_pool(name="heal", bufs=1)),
            ctx.enter_context(tc.tile_pool(name="t3", bufs=1)),
            ctx.enter_context(tc.tile_pool(name="stats", bufs=1)),
            ctx.enter_context(tc.tile_pool(name="epi", bufs=1)),
        )
        if reps == 1:
            emit_body(tc, ctx, pools)
        else:
            with tc.For_i(0, reps, 1):
                emit_body(tc, ctx, pools)

    nc.compile()
    return nc


def _get_nc(reps=1):
    key = ("nc", reps)
    if key not in _NC_CACHE:
        _NC_CACHE[key] = _build_nc(reps)
    return _NC_CACHE[key]


def _make_in_maps(inputs):
    import ml_dtypes

    bf = ml_dtypes.bfloat16
    mut = np.ascontiguousarray(
        np.asarray(inputs["emb_mut"], dtype=np.float32).astype(bf))
    heal = np.ascontiguousarray(
        np.asarray(inputs["emb_heal"], dtype=np.float32).astype(bf))
    epi = np.empty((B, 2 * N + 1), dtype=np.float32)
    epi[:, 0] = np.asarray(inputs["mistake_agent_idx"]).astype(np.float32)
    epi[:, 1:N + 1] = np.asarray(inputs["agent_mask"]).astype(np.float32)
    epi[:, N + 1:] = np.arange(N, dtype=np.float32)[None, :]
    in_maps = []
    for c in range(NCORES):
        sl = slice(c * BP, (c + 1) * BP)
        in_maps.append({
            "emb_mut": mut[sl],
            "emb_heal": heal[sl],
            "epi_in": np.ascontiguousarray(epi[sl]),
        })
    return in_maps


def run_spmd(inputs, trace=False, reps=1):
    """Run on all 8 cores; returns (final_scalar, BassKernelResults)."""
    from concourse.bass_utils import run_bass_kernel_spmd

    nc = _get_nc(reps)
    in_maps = _make_in_maps(inputs)
    res = run_bass_kernel_spmd(nc, in_maps, list(range(NCORES)), trace=trace)
    outs = np.stack([r["out"] for r in res.results])  # [8, 128, 2]
    total = outs[..., 0].sum(dtype=np.float64)
    count = outs[..., 1].sum(dtype=np.float64)
    val = np.float32(total / count) if count > 0 else np.float32(0.0)
    return val, res


def kernel(**inputs) -> np.ndarray:
    val, _ = run_spmd(inputs, trace=False)
    return val
